# revision 4
# baseline (speedup 1.0000x reference)
"""Trainium2 Bass kernel for NeuralConnectionMatrix.

out[i, j] = W2 . relu(R[i, :] + L[j, :] + b1) + b2
  where L = fv @ W1[:, :F].T  (depends on j), R = fv @ W1[:, F:].T (depends on i)

Sharding (8 cores): 2 i-groups x 4 j-groups. Each core computes a
[1024 j, 2048 i] transposed slab:
  - partitions = j (8 blocks of 128), free dim = i (2048)
  - per k: t_k = w2k*relu(x_k) computed as (w2k*R_bcast + w2k*Lb_bias) max/min 0
    (min-0 trick bakes negative w2k signs in host-side)
  - PE accumulates the 16 t_k (+ a b2 slice) into PSUM via identity matmuls
  - ACT drains PSUM -> SBUF, DMA to DRAM
Host precomputes L/R (tiny GEMMs), replicates R across partitions, and
transposes the per-core output slabs back into the full [4096, 4096] array.
"""

import numpy as np

import concourse.bass as bass
import concourse.bacc as bacc
import concourse.mybir as mybir
from concourse.tile import TileContext
from concourse.bass_utils import run_bass_kernel_spmd

N = 4096
F = 3
H = 16
NCORES = 8
IG, JG = 2, 4            # core grid over (i, j)
FI = N // IG             # free-dim (i) extent per core: 2048
PJ = N // JG             # partition-dim (j) extent per core: 1024
NJB = PJ // 128          # j blocks per core: 8
KT = H + 1               # 16 relu slices + 1 b2 slice
NMM = FI // 512          # matmuls per k per j-block (PSUM bank = 512 f32)

FP16 = mybir.dt.float16
FP32 = mybir.dt.float32


def build_bass(act_ks: tuple[int, ...], min_ks: tuple[int, ...]):
    """act_ks: k's computed on ScalarE (must have w2k >= 0).
    min_ks: k's using the min-0 trick on DVE (w2k < 0)."""
    nc = bacc.Bacc()
    rb = nc.dram_tensor("rb", [128, KT, FI], FP16, kind="ExternalInput")
    lbt = nc.dram_tensor("lbt", [NJB, 128, H], FP32, kind="ExternalInput")
    ident = nc.dram_tensor("ident", [128, 128], FP16, kind="ExternalInput")
    outT = nc.dram_tensor("outT", [PJ, FI], FP32, kind="ExternalOutput")

    with TileContext(nc) as tc:
        with (
            tc.tile_pool(name="const", bufs=1) as cpool,
            tc.tile_pool(name="lb", bufs=3) as lbpool,
            tc.tile_pool(name="t", bufs=6) as tpool,
            tc.tile_pool(name="o", bufs=2) as opool,
            tc.tile_pool(name="ps", bufs=2, space="PSUM") as pspool,
        ):
            id_t = cpool.tile([128, 128], FP16, tag="ident")
            nc.sync.dma_start(out=id_t, in_=ident[:, :])
            rbs = []
            for k in range(KT):
                rt = cpool.tile([128, FI], FP16, tag=f"rb{k}")
                nc.sync.dma_start(out=rt, in_=rb[:, k, :])
                rbs.append(rt)

            for jb in range(NJB):
                lb_t = lbpool.tile([128, H], FP32, tag="lb")
                nc.sync.dma_start(out=lb_t, in_=lbt[jb])
                ps = pspool.tile([128, FI], FP32, tag="ps")
                for k in range(H):
                    t = tpool.tile([128, FI], FP16, tag="t")
                    if k in act_ks:
                        nc.scalar.activation(
                            t, rbs[k], mybir.ActivationFunctionType.Relu,
                            bias=lb_t[:, k : k + 1], scale=1.0,
                        )
                    else:
                        op1 = (
                            mybir.AluOpType.min
                            if k in min_ks
                            else mybir.AluOpType.max
                        )
                        nc.vector.tensor_scalar(
                            out=t, in0=rbs[k],
                            scalar1=lb_t[:, k : k + 1], scalar2=0.0,
                            op0=mybir.AluOpType.add, op1=op1,
                        )
                    for nb in range(NMM):
                        nc.tensor.matmul(
                            ps[:, nb * 512 : (nb + 1) * 512],
                            id_t,
                            t[:, nb * 512 : (nb + 1) * 512],
                            start=(k == 0), stop=False,
                        )
                # b2 slice: rb[H] is a constant-b2 tile
                for nb in range(NMM):
                    nc.tensor.matmul(
                        ps[:, nb * 512 : (nb + 1) * 512],
                        id_t,
                        rbs[H][:, nb * 512 : (nb + 1) * 512],
                        start=False, stop=True,
                    )
                ot = opool.tile([128, FI], FP32, tag="o")
                nc.scalar.copy(ot, ps)
                nc.sync.dma_start(
                    out=outT[jb * 128 : (jb + 1) * 128, :], in_=ot
                )
    nc.finalize()
    return nc


def kernel(feature_vectors, W1, b1, W2, b2):
    fv = np.asarray(feature_vectors, dtype=np.float32)
    W1 = np.asarray(W1, dtype=np.float32)
    b1 = np.asarray(b1, dtype=np.float32)
    W2 = np.asarray(W2, dtype=np.float32)
    b2 = np.asarray(b2, dtype=np.float32)

    L = fv @ W1[:, :F].T + b1        # [N, H], j side (bias, on partitions)
    R = fv @ W1[:, F:].T             # [N, H], i side (free dim)
    w2 = W2[0]                       # [H]
    b2v = float(b2[0])

    # Fold w2 into both operands; negative w2k handled with min-0 trick.
    Rs = R * w2[None, :]             # [N, H]
    Ls = L * w2[None, :]             # [N, H]

    min_ks = tuple(int(k) for k in range(H) if w2[k] < 0)
    # ScalarE helper: take some positive-w2 k's off the DVE.
    pos_ks = [int(k) for k in range(H) if w2[k] >= 0]
    act_ks = tuple(pos_ks[:3])

    nc = build_bass(act_ks, min_ks)

    ident = np.eye(128, dtype=np.float16)
    in_maps = []
    for c in range(NCORES):
        ig, jg = divmod(c, JG)
        isl = slice(ig * FI, (ig + 1) * FI)
        jsl = slice(jg * PJ, (jg + 1) * PJ)
        base = np.concatenate(
            [Rs[isl, :].T, np.full((1, FI), b2v, dtype=np.float32)], axis=0
        ).astype(np.float16)        # [KT, FI]
        rb_c = np.ascontiguousarray(
            np.broadcast_to(base[None, :, :], (128, KT, FI))
        )
        lbt_c = np.ascontiguousarray(
            Ls[jsl, :].reshape(NJB, 128, H)
        )
        in_maps.append({"rb": rb_c, "lbt": lbt_c, "ident": ident})

    res = run_bass_kernel_spmd(nc, in_maps, core_ids=list(range(NCORES)))

    out = np.empty((N, N), dtype=np.float32)
    for c in range(NCORES):
        ig, jg = divmod(c, JG)
        out[ig * FI : (ig + 1) * FI, jg * PJ : (jg + 1) * PJ] = (
            res.results[c]["outT"].T
        )
    return out
